# revision 40
# baseline (speedup 1.0000x reference)
"""EntityAttentionLayer on 8 trn2 NeuronCores.

Strategy (entity-sharded, per the sharding hint):
  - Host (numpy): gather mention start/end encodings, q-projection
    (mention_encodings output), mention/entity norms, one-hot scatter
    matrices, input transposes/casts. All cheap input prep.
  - Device (SPMD x8, entity dim sharded 4000/core):
      scores twice (fp16 matmul): [m,e] orientation for cos/attn output
      tiles, [e,m] orientation feeding exp -> U = exp@ent (bf16 matmul)
      and sumexp. One packed AllReduce per 512-mention block carries
      U^T [256,512] + sumexp [512]. attn = exp * (1/sumexp_total).
      Tail: retrieved = (U/s) @ we + be, scatter-add into core's own
      batch via one-hot matmul (handles duplicate positions), LayerNorm,
      write encoded slice.
  - Outputs assembled on host: attn/cos concat on entity axis, encoded
    stacked on batch axis.
"""

import os
from contextlib import ExitStack

import numpy as np
import ml_dtypes

_STAGE = int(os.environ.get("K_STAGE", "7"))
_GATE = _STAGE if _STAGE < 10 else _STAGE // 10

import concourse.bass as bass
import concourse.bacc as bacc
import concourse.mybir as mybir
import concourse.tile as tile
from concourse.bass_utils import run_bass_kernel_spmd

F32 = mybir.dt.float32
F16 = mybir.dt.float16
BF16 = mybir.dt.bfloat16
I16 = mybir.dt.int16
Alu = mybir.AluOpType
Act = mybir.ActivationFunctionType
AX = mybir.AxisListType

NC = 8
B, T, H = 8, 2048, 768
M, D, E = 4096, 256, 32000
ES = E // NC          # 4000 entities per core
NB = 8                # AllReduce blocks
MBLK = M // NB        # 512 mentions per block
ETW = 500             # m-orientation entity tile width (8 * 500 = 4000)
SLOTS = 1024          # padded per-batch mention slots
_SMALL = 1e-8
_LN_EPS = 1e-12

_CACHED_NC = None


def _build_device_kernel():
    nc = bacc.Bacc("TRN2", debug=False, target_bir_lowering=False, num_devices=NC)

    # --- I/O ---
    qt_d = nc.dram_tensor("qt16", [256, M], F16, kind="ExternalInput")
    entt_d = nc.dram_tensor("entt16", [256, ES], F16, kind="ExternalInput")
    entb_d = nc.dram_tensor("entb16", [128, 32, 256], BF16, kind="ExternalInput")
    re_d = nc.dram_tensor("recip_e_bc", [128, ES], F32, kind="ExternalInput")
    rm_d = nc.dram_tensor("recip_m", [128, 32], F32, kind="ExternalInput")
    mask_d = nc.dram_tensor("mask_t", [128, 32], F32, kind="ExternalInput")
    we_d = nc.dram_tensor("we16", [256, H], BF16, kind="ExternalInput")
    row3_d = nc.dram_tensor("row3b", [128, 3, H], F32, kind="ExternalInput")
    pscat_d = nc.dram_tensor("pscat", [16, 128, 8, 128], BF16, kind="ExternalInput")
    idx_d = nc.dram_tensor("idxs", [128, 64], I16, kind="ExternalInput")
    encin_d = nc.dram_tensor("enc_in", [T, H], F32, kind="ExternalInput")

    attn_d = nc.dram_tensor("attn_out", [M, ES], F32, kind="ExternalOutput")
    cos_d = nc.dram_tensor("cos_out", [M, ES], F32, kind="ExternalOutput")
    enc_d = nc.dram_tensor("enc_out", [T, H], F32, kind="ExternalOutput")

    rg = [list(range(NC))]

    with tile.TileContext(nc) as tc:
        with (
            tc.tile_pool(name="const", bufs=1) as cp,
            tc.tile_pool(name="dram", bufs=1, space="DRAM") as dp,
            tc.tile_pool(name="exp", bufs=5) as expp,
            tc.tile_pool(name="stage", bufs=3) as stp,
            tc.tile_pool(name="small", bufs=4) as smp,
            tc.tile_pool(name="srows", bufs=2) as srp,
        ):
            # ---- main-phase constants (pool closes before tail) ----
            main_ps = ExitStack()
            mcp = main_ps.enter_context(tc.tile_pool(name="mainconst", bufs=1))
            qt_sb = mcp.tile([128, 2, M], F16, name="qt_sb")
            entt_sb = mcp.tile([128, 2, ES], F16, name="entt_sb")
            for k in range(2):
                nc.sync.dma_start(out=qt_sb[:, k, :], in_=qt_d[k * 128:(k + 1) * 128, :])
                nc.sync.dma_start(out=entt_sb[:, k, :], in_=entt_d[k * 128:(k + 1) * 128, :])
            entb_sb = mcp.tile([128, 32, 256], BF16, name="entb_sb")
            nc.sync.dma_start(out=entb_sb[:], in_=entb_d[:])
            re_bc = mcp.tile([128, ES], F32, name="re_bc")
            nc.sync.dma_start(out=re_bc[:], in_=re_d[:])
            rm_sb = mcp.tile([128, 32], F32, name="rm_sb")
            nc.sync.dma_start(out=rm_sb[:], in_=rm_d[:])

            # ---- persistent constants ----
            mask_sb = cp.tile([128, 32], F32, name="mask_sb")
            nc.sync.dma_start(out=mask_sb[:], in_=mask_d[:])
            we_sb = cp.tile([128, 2, H], BF16, name="we_sb")
            for k in range(2):
                nc.sync.dma_start(out=we_sb[:, k, :], in_=we_d[k * 128:(k + 1) * 128, :])
            r3_sb = cp.tile([128, 3, H], F32, name="r3_sb")
            nc.sync.dma_start(out=r3_sb[:], in_=row3_d[:])
            be_bc = r3_sb[:, 0, :]
            lns_bc = r3_sb[:, 1, :]
            lnb_bc = r3_sb[:, 2, :]
            idx_sb = cp.tile([128, 64], I16, name="idx_sb")
            nc.sync.dma_start(out=idx_sb[:], in_=idx_d[:])
            eps_t = cp.tile([128, 1], F32, name="eps_t")
            nc.vector.memset(eps_t[:], _LN_EPS)

            # ---- internal DRAM (tracked tiles) ----
            ar_in = [dp.tile([257, MBLK], F32, name=f"ar_in{b}") for b in range(NB)]
            ar_out = [
                dp.tile([257, MBLK], F32, name=f"ar_out{b}", addr_space="Shared")
                for b in range(NB)
            ]
            retrh = dp.tile([M, H], BF16, name="retrh")

            exp_tiles = {}
            rcp_tiles = {}

            # ================= main loop over AllReduce blocks =================
            psm = main_ps.enter_context(tc.tile_pool(name="psm", bufs=2, space="PSUM"))
            pstp = main_ps.enter_context(tc.tile_pool(name="pst", bufs=2, space="PSUM"))
            psup = main_ps.enter_context(tc.tile_pool(name="psu", bufs=1, space="PSUM"))
            for b in range(NB):
                # --- m-orientation: cos output + exp (retained) + rowsum ---
                for mt in range(4):
                    mg = b * 4 + mt
                    exp_t = expp.tile([128, ES], BF16, tag="exp", name=f"exp_{mg}")
                    exp_tiles[mg] = exp_t
                    for et in range(8):
                        ps = psm.tile([128, ETW], F32, tag="psm", name=f"psm_{mg}_{et}")
                        for k in range(2):
                            nc.tensor.matmul(
                                ps[:],
                                lhsT=qt_sb[:, k, mg * 128:(mg + 1) * 128],
                                rhs=entt_sb[:, k, et * ETW:(et + 1) * ETW],
                                start=(k == 0),
                                stop=(k == 1),
                            )
                        cos_t = stp.tile([128, ETW], F32, tag="cos", name=f"cos_{mg}_{et}", bufs=2)
                        nc.vector.scalar_tensor_tensor(
                            out=cos_t[:],
                            in0=ps[:],
                            scalar=rm_sb[:, mg:mg + 1],
                            in1=re_bc[:, et * ETW:(et + 1) * ETW],
                            op0=Alu.mult,
                            op1=Alu.mult,
                        )
                        nc.sync.dma_start(
                            out=cos_d[mg * 128:(mg + 1) * 128, et * ETW:(et + 1) * ETW],
                            in_=cos_t[:],
                        )
                        nc.scalar.activation(
                            out=exp_t[:, et * ETW:(et + 1) * ETW], in_=ps[:], func=Act.Exp
                        )
                    sp = smp.tile([128, 1], F32, tag="spart", name=f"sp_{mg}")
                    nc.vector.reduce_sum(sp[:], exp_t[:], AX.X)
                    nc.sync.dma_start(
                        out=ar_in[b][256:257, mt * 128:(mt + 1) * 128], in_=sp[:]
                    )

                # --- e-orientation: exp^T -> U accumulation ---
                psu0 = psup.tile([128, MBLK], F32, tag="psu0", name=f"psu0_{b}")
                psu1 = psup.tile([128, MBLK], F32, tag="psu1", name=f"psu1_{b}")
                psu = (psu0, psu1)
                for j in range(32):
                    ej = 128 if j < 31 else 32
                    pst = pstp.tile([128, MBLK], F32, tag="pst", name=f"pst_{b}_{j}")
                    for k in range(2):
                        nc.tensor.matmul(
                            pst[:ej, :],
                            lhsT=entt_sb[:, k, j * 128:j * 128 + ej],
                            rhs=qt_sb[:, k, b * MBLK:(b + 1) * MBLK],
                            start=(k == 0),
                            stop=(k == 1),
                        )
                    ext = stp.tile([128, MBLK], BF16, tag="ext", name=f"ext_{b}_{j}")
                    nc.scalar.activation(out=ext[:ej, :], in_=pst[:ej, :], func=Act.Exp)
                    for h in range(2):
                        nc.tensor.matmul(
                            psu[h][:],
                            lhsT=entb_sb[:ej, j, h * 128:(h + 1) * 128],
                            rhs=ext[:ej, :],
                            start=(j == 0),
                            stop=(j == 31),
                        )
                for h in range(2):
                    ut = stp.tile([128, MBLK], F32, tag="ut", name=f"ut_{b}_{h}")
                    nc.vector.tensor_copy(out=ut[:], in_=psu[h][:])
                    nc.sync.dma_start(
                        out=ar_in[b][h * 128:(h + 1) * 128, :], in_=ut[:]
                    )

                # --- AllReduce: packed U^T [256,512] + sumexp row [1,512] ---
                nc.gpsimd.collective_compute(
                    "AllReduce",
                    Alu.add,
                    replica_groups=rg,
                    ins=[ar_in[b][:, :]],
                    outs=[ar_out[b][:, :]],
                )

                # --- attn = exp * 1/s_total ---
                for mt in range(4):
                    mg = b * 4 + mt
                    s_mt = smp.tile([128, 1], F32, tag="stot", name=f"stot_{mg}")
                    nc.sync.dma_start(
                        out=s_mt[:], in_=ar_out[b][256:257, mt * 128:(mt + 1) * 128]
                    )
                    rcp = smp.tile([128, 1], F32, tag="rcp", name=f"rcp_{mg}")
                    nc.vector.reciprocal(out=rcp[:], in_=s_mt[:])
                    rcp_tiles[mg] = rcp
                    exp_t = exp_tiles[mg]
                    for et in range(8):
                        at = stp.tile([128, ETW], F32, tag="at", name=f"at_{mg}_{et}", bufs=2)
                        nc.vector.tensor_scalar_mul(
                            out=at[:], in0=exp_t[:, et * ETW:(et + 1) * ETW], scalar1=rcp[:]
                        )
                        if _STAGE != 26:
                            nc.sync.dma_start(
                                out=attn_d[mg * 128:(mg + 1) * 128, et * ETW:(et + 1) * ETW],
                                in_=at[:],
                            )

            main_ps.close()

            # ================= tail: retrieved / scatter / layernorm =================
            with (
                tc.tile_pool(name="tailps", bufs=2, space="PSUM") as tps,
                tc.tile_pool(name="tail", bufs=3) as tlp,
            ):
                # unscaled U^T (bf16) for all blocks: [128, d-half, block, 512]
                utall = tlp.tile([128, 2, NB, MBLK], BF16, name="utall", bufs=1)
                for b in range(NB if _GATE >= 2 else 0):
                    for h in range(2):
                        utf = tlp.tile([128, MBLK], F32, tag="utf", name=f"utf_{b}_{h}", bufs=2)
                        nc.sync.dma_start(
                            out=utf[:], in_=ar_out[b][h * 128:(h + 1) * 128, :]
                        )
                        nc.vector.tensor_copy(out=utall[:, h, b, :], in_=utf[:])

                # retrieved_H = (U@we)/s + be, masked; spill bf16 for gather
                for mg in range(32 if _GATE >= 2 else 0):
                    b, mt = mg // 4, mg % 4
                    psr = [
                        tps.tile([128, 384], F32, tag=f"psr{nh}", name=f"psr{nh}_{mg}")
                        for nh in range(2)
                    ]
                    for nh in range(2):
                        for h in range(2):
                            nc.tensor.matmul(
                                psr[nh][:],
                                lhsT=utall[:, h, b, mt * 128:(mt + 1) * 128],
                                rhs=we_sb[:, h, nh * 384:(nh + 1) * 384],
                                start=(h == 0),
                                stop=(h == 1),
                            )
                    s_mg = srp.tile([128, 1], F32, tag="s_mg", name=f"s_mg_{mg}")
                    nc.sync.dma_start(
                        out=s_mg[:], in_=ar_out[b][256:257, mt * 128:(mt + 1) * 128]
                    )
                    rcp_mg = srp.tile([128, 1], F32, tag="rcp_mg", name=f"rcp_mg_{mg}")
                    nc.vector.reciprocal(out=rcp_mg[:], in_=s_mg[:])
                    rtmp = tlp.tile([128, H], F32, tag="rtmp", name=f"rtmp_{mg}", bufs=2)
                    for nh in range(2):
                        nc.vector.scalar_tensor_tensor(
                            out=rtmp[:, nh * 384:(nh + 1) * 384], in0=psr[nh][:],
                            scalar=rcp_mg[:], in1=be_bc[:, nh * 384:(nh + 1) * 384],
                            op0=Alu.mult, op1=Alu.add,
                        )
                    rh = tlp.tile([128, H], BF16, tag="rh", name=f"rh_{mg}", bufs=2)
                    nc.vector.tensor_scalar_mul(out=rh[:], in0=rtmp[:], scalar1=mask_sb[:, mg:mg + 1])
                    nc.sync.dma_start(out=retrh[mg * 128:(mg + 1) * 128, :], in_=rh[:])

                if _STAGE == 26:
                    # debug: dump raw AllReduced buffers into attn output
                    for b in range(NB):
                        nc.sync.dma_start(
                            out=attn_d[b * 257:(b + 1) * 257, 0:MBLK], in_=ar_out[b][:, :]
                        )
                if _STAGE == 25:
                    # debug: dump retrh (bf16) into cos output cols 0:768
                    for mg in range(32):
                        dbg = tlp.tile([128, H], BF16, tag="dbg", name=f"dbg_{mg}", bufs=2)
                        nc.sync.dma_start(out=dbg[:], in_=retrh[mg * 128:(mg + 1) * 128, :])
                        dbgf = tlp.tile([128, H], F32, tag="dbgf", name=f"dbgf_{mg}", bufs=2)
                        nc.vector.tensor_copy(out=dbgf[:], in_=dbg[:])
                        nc.sync.dma_start(
                            out=cos_d[mg * 128:(mg + 1) * 128, 0:H], in_=dbgf[:]
                        )

                # gather own-batch mention rows into slot order
                rperm = tlp.tile([128, 8, H], BF16, name="rperm", bufs=1)
                if _GATE >= 3:
                    nc.gpsimd.dma_gather(
                        out_ap=rperm[:],
                        in_ap=retrh[:, :],
                        idxs_ap=idx_sb[:],
                        num_idxs=SLOTS,
                        num_idxs_reg=SLOTS,
                        elem_size=H,
                    )

                if _STAGE == 35:
                    # debug: dump rperm into enc_out rows 0:1024 (row p*8+j)
                    rpf = tlp.tile([128, 8, H], F32, name="rpf", bufs=1)
                    nc.vector.tensor_copy(out=rpf[:], in_=rperm[:])
                    nc.sync.dma_start(out=enc_d[0:1024, :], in_=rpf[:])

                # scatter-add (one-hot matmul) + residual + layernorm
                for tt in range(16 if _GATE >= 4 else 0):
                    pstile = tlp.tile([128, 8, 128], BF16, tag="pscat", name=f"pscat_{tt}", bufs=2)
                    nc.sync.dma_start(out=pstile[:], in_=pscat_d[tt])
                    psd = [
                        tps.tile([128, 384], F32, tag=f"psd{nh}", name=f"psd{nh}_{tt}")
                        for nh in range(2)
                    ]
                    for nh in range(2):
                        for j in range(8):
                            nc.tensor.matmul(
                                psd[nh][:],
                                lhsT=pstile[:, j, :],
                                rhs=rperm[:, j, nh * 384:(nh + 1) * 384],
                                start=(j == 0),
                                stop=(j == 7),
                            )
                    ei = tlp.tile([128, H], F32, tag="ei", name=f"ei_{tt}", bufs=2)
                    nc.sync.dma_start(out=ei[:], in_=encin_d[tt * 128:(tt + 1) * 128, :])
                    x = tlp.tile([128, H], F32, tag="x", name=f"x_{tt}", bufs=2)
                    for nh in range(2):
                        nc.vector.tensor_tensor(
                            out=x[:, nh * 384:(nh + 1) * 384], in0=psd[nh][:],
                            in1=ei[:, nh * 384:(nh + 1) * 384], op=Alu.add)
                    out_t = x
                    if _GATE >= 5:
                        mu = smp.tile([128, 1], F32, tag="mu", name=f"mu_{tt}")
                        nc.vector.reduce_sum(mu[:], x[:], AX.X)
                        mus = smp.tile([128, 1], F32, tag="mus", name=f"mus_{tt}")
                        nc.scalar.activation(out=mus[:], in_=mu[:], func=Act.Copy, scale=1.0 / H)
                        xc = tlp.tile([128, H], F32, tag="xc", name=f"xc_{tt}", bufs=2)
                        nc.vector.tensor_scalar_sub(out=xc[:], in0=x[:], scalar1=mus[:])
                        out_t = xc
                    if _GATE >= 6:
                        # reuse x as the squared buffer
                        nc.vector.tensor_tensor(out=x[:], in0=xc[:], in1=xc[:], op=Alu.mult)
                        vr = smp.tile([128, 1], F32, tag="vr", name=f"vr_{tt}")
                        nc.vector.reduce_sum(vr[:], x[:], AX.X)
                        sd = smp.tile([128, 1], F32, tag="sd", name=f"sd_{tt}")
                        nc.scalar.activation(
                            out=sd[:], in_=vr[:], func=Act.Sqrt, scale=1.0 / H, bias=eps_t[:]
                        )
                        rstd = smp.tile([128, 1], F32, tag="rstd", name=f"rstd_{tt}")
                        nc.vector.reciprocal(out=rstd[:], in_=sd[:])
                        nc.vector.tensor_scalar_mul(out=xc[:], in0=xc[:], scalar1=rstd[:])
                    if _GATE >= 7:
                        nc.vector.tensor_tensor(out=xc[:], in0=xc[:], in1=lns_bc, op=Alu.mult)
                        nc.vector.tensor_tensor(out=xc[:], in0=xc[:], in1=lnb_bc, op=Alu.add)
                    nc.sync.dma_start(out=enc_d[tt * 128:(tt + 1) * 128, :], in_=out_t[:])

    nc.compile()
    return nc


def _get_nc():
    global _CACHED_NC
    if _CACHED_NC is None:
        _CACHED_NC = _build_device_kernel()
    return _CACHED_NC


def _prep_inputs(
    encoded_input, mention_batch_positions, mention_start_positions,
    mention_end_positions, mention_mask, entity_embeddings,
    wq, bq, we, be, ln_scale, ln_bias,
):
    enc = np.asarray(encoded_input, np.float32)
    bpos = np.asarray(mention_batch_positions)
    spos = np.asarray(mention_start_positions)
    epos = np.asarray(mention_end_positions)
    ent = np.asarray(entity_embeddings, np.float32)

    start_enc = enc[bpos, spos]
    end_enc = enc[bpos, epos]
    q = (np.concatenate([start_enc, end_enc], axis=-1) @ np.asarray(wq, np.float32)
         + np.asarray(bq, np.float32)).astype(np.float32)

    m_norm = np.linalg.norm(q, axis=-1)
    e_norm = np.linalg.norm(ent, axis=-1)
    recip_m = (1.0 / (_SMALL + m_norm)).astype(np.float32)
    recip_e_full = (1.0 / (_SMALL + e_norm)).astype(np.float32)

    qt16 = np.ascontiguousarray(q.T).astype(np.float16)
    rm_t = np.ascontiguousarray(recip_m.reshape(32, 128).T)  # [128, 32]
    mask_t = np.ascontiguousarray(
        np.asarray(mention_mask, np.float32).reshape(32, 128).T)
    we16 = np.asarray(we, np.float32).astype(ml_dtypes.bfloat16)
    row3 = np.stack([
        np.asarray(be, np.float32),
        np.asarray(ln_scale, np.float32),
        np.asarray(ln_bias, np.float32),
    ]).astype(np.float32)
    row3b = np.ascontiguousarray(np.broadcast_to(row3[None], (128, 3, H)))

    in_maps = []
    for c in range(NC):
        sl = slice(c * ES, (c + 1) * ES)
        ent_c = ent[sl]
        entt16 = np.ascontiguousarray(ent_c.T).astype(np.float16)
        entp = np.zeros((32 * 128, D), np.float32)
        entp[:ES] = ent_c
        entb16 = np.ascontiguousarray(
            entp.reshape(32, 128, D).transpose(1, 0, 2)).astype(ml_dtypes.bfloat16)

        mentions_c = np.nonzero(bpos == c)[0]
        n_c = len(mentions_c)
        assert n_c <= SLOTS, f"batch {c} has {n_c} mentions > {SLOTS} slots"
        idx_flat = np.zeros(SLOTS, np.int16)
        idx_flat[:n_c] = mentions_c.astype(np.int16)
        idx_arr = np.zeros((128, 64), np.int16)
        idx_arr[:16, :] = idx_flat.reshape(64, 16).T

        pscat = np.zeros((16, 128, 8, 128), np.float32)
        for s in range(n_c):
            t = int(spos[mentions_c[s]])
            pscat[t // 128, s % 128, s // 128, t % 128] += 1.0
        pscat16 = pscat.astype(ml_dtypes.bfloat16)

        in_maps.append({
            "qt16": qt16,
            "entt16": entt16,
            "entb16": entb16,
            "recip_e_bc": np.ascontiguousarray(
                np.broadcast_to(recip_e_full[sl][None], (128, ES))),
            "recip_m": rm_t,
            "mask_t": mask_t,
            "we16": we16,
            "row3b": row3b,
            "pscat": pscat16,
            "idxs": idx_arr,
            "enc_in": np.ascontiguousarray(enc[c]),
        })
    return q, in_maps


def kernel(**inputs):
    q, in_maps = _prep_inputs(**inputs)
    nc = _get_nc()
    res = run_bass_kernel_spmd(nc, in_maps, core_ids=list(range(NC)))
    outs = res.results
    attn = np.concatenate([outs[c]["attn_out"] for c in range(NC)], axis=1)
    cos = np.concatenate([outs[c]["cos_out"] for c in range(NC)], axis=1)
    encoded = np.stack([outs[c]["enc_out"] for c in range(NC)], axis=0)
    return encoded, q, cos, attn
